# revision 51
# baseline (speedup 1.0000x reference)
"""Trainium2 Bass kernel for the HPNET loss (confidence + depth + rotation).

Contract: kernel(**inputs) takes the FULL unsharded fp32 inputs and returns
the full output (tuple of three f32 scalars), distributing across 8 cores.

Strategy (raw Bass, no TileContext):
  - Data-parallel: batch dim of confidence/gt/weight and ROI dim of
    depth_and_rotation/ann_* split 8 ways; tiny [128, 3] partials per core
    ([conf (partition 0 only), depth, rot]) are reduced on host.
  - The confidence stream (a, b, w) is host-cast to fp8 e4m3 and packed into
    ONE chunk-interleaved DRAM tensor per core: per chunk [a|b|w]. SWDGE
    (gpsimd) chunk DMAs cast fp8 -> bf16 on the way into SBUF, so HBM reads
    are quartered and the stream runs at the SBUF-write fabric rate
    (~413 GB/s), which also makes the stream time run-to-run stable.
    Quantization adds ~1.2e-3 rel err vs the 2e-2 budget.
  - No SBUF buffer reuse: the whole 96 KiB/partition stream is resident, so
    every DMA is issued up-front with zero recycle waits and the queue
    streams back-to-back.
  - Compute per chunk: DVE sub (in-place, bf16 2x mode), ACT square
    (in-place), DVE w*d^2 (2x mode), then the otherwise-idle PE reduces
    wd2 columns into a PSUM row via ones-vector matmuls (accumulated across
    all chunks); one final DVE reduce of the PSUM row yields the conf sum.
    Chunk sizes taper at the end so the post-last-DMA drain is short.
  - ROI losses (fp32, tiny) ride the first DMA on the sync-engine ring
    (heading the ring avoids starvation behind the stream) and overlap the
    start of the stream.  m_gt normalization and min(n1,n2) =
    sqrt(min(n1sq,n2sq)) keep the ROI chain to two ACT ops.
  - Raw-Bass sync protocol (one sem update per instruction max): every DVE op
    increments the retire-counter sem `vq`, every ACT compute op `aq`, every
    PE matmul `pq`.  All cross-engine dependencies are thresholds on these
    counters (= producer's position in its engine's program).  Same-engine
    RAW hazards in the ROI chain are ordered by waiting on `vq` (TRN2 engines
    do not interlock back-to-back instructions); the streaming chunk ops are
    ordered transitively through their cross-engine waits.  Each chunk DMA
    gets a dedicated completion sem (+16).  All sems are cleared after the
    final barrier so NEFF re-executions start clean.
"""

import numpy as np

_NCORES = 8
_B = 256
_HW = 256 * 256
_N = 8192
_PB = _B // _NCORES            # batches per core
_F = _PB * _HW // 128          # 16384 free elems per partition
_CHUNKS = (1024, 2048, 2048, 2048, 2048, 2048, 2048, 1024, 1024,
           512, 256, 128, 128)
assert sum(_CHUNKS) == _F
_NCH = len(_CHUNKS)
_NTAIL = 2                     # last chunks: DVE-only square, PSUM group B
_R = _N // _NCORES // 128      # 8 ROIs per partition
_OUTC = 4                      # [confA, confB (partition 0 only), depth, rot]
_ROIW = _R * 5 * 2 + _R        # dr(40) + ann(40) + msk(8) = 88 f32
_TW = 256                      # PE column-sum tile width (half a PSUM bank)
_FP8 = True                    # store conf stream as fp8 e4m3 in HBM,
                               # SWDGE-cast to bf16 on the way into SBUF

_CACHE = {}


class _Counter:
    """Emit ops on one engine; every op .then_inc's the engine's retire
    counter sem. `serialize=True` additionally waits for all previously
    emitted ops on this engine (same-engine memory ordering)."""

    def __init__(self, eng, sem):
        self.eng, self.sem, self.n = eng, sem, 0

    def op(self, f, *a, serialize=False, **k):
        if serialize and self.n:
            self.eng.wait_ge(self.sem, self.n)
        ins = f(*a, **k)
        ins.then_inc(self.sem, 1)
        self.n += 1
        return ins


def _emit_quat2mat(v, nc, st, f32, q, m, width):
    """Emit rotation-matrix entries (column-major: m[:,:,3*col+row]) for
    quaternions given as 4 APs of shape [128, width]. No normalization.
    All ops serialized on the DVE stream (RAW chains throughout)."""
    eng = v.eng
    sq = st.enter_context(nc.sbuf_tensor([128, 4, width], f32))
    for i in range(4):
        v.op(eng.tensor_mul, sq[:, i, :], q[i], q[i], serialize=True)
    qd = st.enter_context(nc.sbuf_tensor([128, 3, width], f32))
    for a0 in range(3):
        v.op(eng.tensor_scalar_mul, qd[:, a0, :], q[a0], 2.0, serialize=True)
    pairs = [(0, 1), (0, 2), (0, 3), (1, 2), (1, 3), (2, 3)]
    pp = st.enter_context(nc.sbuf_tensor([128, 6, width], f32))
    for k, (x, y) in enumerate(pairs):
        v.op(eng.tensor_mul, pp[:, k, :], qd[:, x, :], q[y], serialize=True)
    uv = st.enter_context(nc.sbuf_tensor([128, 4, width], f32))
    v.op(eng.tensor_sub, uv[:, 0, :], sq[:, 0, :], sq[:, 3, :],
         serialize=True)
    v.op(eng.tensor_sub, uv[:, 1, :], sq[:, 1, :], sq[:, 2, :],
         serialize=True)
    v.op(eng.tensor_add, uv[:, 2, :], sq[:, 0, :], sq[:, 3, :],
         serialize=True)
    v.op(eng.tensor_add, uv[:, 3, :], sq[:, 1, :], sq[:, 2, :],
         serialize=True)
    P01, P02, P03, P12, P13, P23 = (pp[:, k, :] for k in range(6))
    u, vv, u2, v2 = (uv[:, k, :] for k in range(4))
    v.op(eng.tensor_add, m[:, :, 0], u, vv, serialize=True)
    v.op(eng.tensor_add, m[:, :, 1], P12, P03, serialize=True)
    v.op(eng.tensor_sub, m[:, :, 2], P13, P02, serialize=True)
    v.op(eng.tensor_sub, m[:, :, 3], P12, P03, serialize=True)
    v.op(eng.tensor_sub, m[:, :, 4], u, vv, serialize=True)
    v.op(eng.tensor_add, m[:, :, 5], P23, P01, serialize=True)
    v.op(eng.tensor_add, m[:, :, 6], P13, P02, serialize=True)
    v.op(eng.tensor_sub, m[:, :, 7], P23, P01, serialize=True)
    v.op(eng.tensor_sub, m[:, :, 8], u2, v2, serialize=True)


def build_nc():
    from contextlib import ExitStack
    import concourse.bacc as bacc
    import concourse.mybir as mybir

    f32 = mybir.dt.float32
    bf16 = mybir.dt.bfloat16
    Alu = mybir.AluOpType
    Act = mybir.ActivationFunctionType
    AxX = mybir.AxisListType.X

    nc = bacc.Bacc("TRN2", target_bir_lowering=False, debug=False,
                   num_devices=_NCORES)

    sdt = mybir.dt.float8e4 if _FP8 else bf16
    comb = nc.dram_tensor("comb", [128, 3 * _F], sdt, kind="ExternalInput")
    roid = nc.dram_tensor("roid", [128, _ROIW], f32, kind="ExternalInput")
    out = nc.dram_tensor("out", [128, _OUTC], f32, kind="ExternalOutput")

    offs = []
    o = 0
    for ch in _CHUNKS:
        offs.append(o)
        o += ch

    # DVE program positions (1-based vq thresholds), computed as we emit.
    pos = {}

    with ExitStack() as st:
        sb = st.enter_context(nc.sbuf_tensor([128, 3 * _F], bf16))
        rb = st.enter_context(nc.sbuf_tensor([128, _ROIW], f32))
        accs = st.enter_context(nc.sbuf_tensor([128, _OUTC], f32))

        # ROI scratch (all fp32, tiny)
        W2 = 2 * _R
        qsq = st.enter_context(nc.sbuf_tensor([128, _R, 4], f32))
        nrm2 = st.enter_context(nc.sbuf_tensor([128, _R], f32))
        nrm = st.enter_context(nc.sbuf_tensor([128, _R], f32))
        rinv = st.enter_context(nc.sbuf_tensor([128, _R], f32))
        Q = st.enter_context(nc.sbuf_tensor([128, 4, W2], f32))
        M = st.enter_context(nc.sbuf_tensor([128, W2, 9], f32))
        d1 = st.enter_context(nc.sbuf_tensor([128, _R, 9], f32))
        d1s = st.enter_context(nc.sbuf_tensor([128, _R, 9], f32))
        n1sq = st.enter_context(nc.sbuf_tensor([128, _R], f32))
        f2 = st.enter_context(nc.sbuf_tensor([128, _R, 9], f32))
        f2s = st.enter_context(nc.sbuf_tensor([128, _R, 9], f32))
        n2sq = st.enter_context(nc.sbuf_tensor([128, _R], f32))
        nminsq = st.enter_context(nc.sbuf_tensor([128, _R], f32))
        nmin = st.enter_context(nc.sbuf_tensor([128, _R], f32))
        dd = st.enter_context(nc.sbuf_tensor([128, _R], f32))
        dd2 = st.enter_context(nc.sbuf_tensor([128, _R], f32))
        dscr = st.enter_context(nc.sbuf_tensor([128, _R], f32))
        rscr = st.enter_context(nc.sbuf_tensor([128, _R], f32))

        ones = st.enter_context(nc.sbuf_tensor([128, 1], bf16))
        ps = st.enter_context(nc.psum_tensor([1, _TW], f32))
        psb = st.enter_context(nc.psum_tensor([1, max(_CHUNKS[-_NTAIL:])], f32))

        dsems = [nc.alloc_semaphore(f"dsem{i}") for i in range(_NCH)]
        rsem = nc.alloc_semaphore("rsem")   # ROI DMA completion
        fsem = nc.alloc_semaphore("fsem")   # out DMA done
        vq = nc.alloc_semaphore("vq")       # DVE retire counter
        aq = nc.alloc_semaphore("aq")       # ACT retire counter
        pq = nc.alloc_semaphore("pq")       # PE retire counter
        all_sems = dsems + [rsem, fsem, vq, aq, pq]

        n_mm_a = sum((ch + _TW - 1) // _TW for ch in _CHUNKS[:-_NTAIL])
        n_mm = n_mm_a + _NTAIL

        # ---- vector program (emitted first so `pos` is known to others) ----
        with nc.Block(no_gpsimd_drain=not _FP8) as blk:

            @blk.vector
            def _(eng):
                v = _Counter(eng, vq)
                dr3 = rb[:, 0:5 * _R].rearrange("p (r c) -> p r c", c=5)
                an3 = rb[:, 5 * _R:10 * _R].rearrange("p (r c) -> p r c", c=5)
                mt = rb[:, 10 * _R:11 * _R]

                v.op(eng.memset, ones[:], 1.0)   # PE stationary ones vector
                v.op(eng.memset, accs[:], 0.0)

                eng.wait_ge(rsem, 16)
                # depth loss (DVE only; serialized RAW chain)
                v.op(eng.tensor_sub, dd[:], dr3[:, :, 0], an3[:, :, 0])
                v.op(eng.tensor_mul, dd2[:], dd[:], dd[:], serialize=True)
                v.op(eng.scalar_tensor_tensor,
                     out=dscr[:], in0=dd2[:], scalar=1.0, in1=mt,
                     op0=Alu.mult, op1=Alu.mult, serialize=True,
                     accum_out=accs[:, 2:3])

                # rotation part A: |q|^2 of predicted quaternion
                v.op(eng.tensor_mul, qsq[:], dr3[:, :, 1:5], dr3[:, :, 1:5])
                v.op(eng.tensor_reduce, out=nrm2[:], in_=qsq[:], axis=AxX,
                     op=Alu.add, serialize=True)
                pos["nrm2"] = v.n

                # chunk 0's sub comes BEFORE the long rotation chain so the
                # ACT square pipeline starts ~7us earlier; the whole drain
                # cascade at stream-end shifts left accordingly.
                eng.wait_ge(dsems[0], 16)
                ch0 = _CHUNKS[0]
                v.op(eng.tensor_sub, sb[:, 0:ch0], sb[:, 0:ch0],
                     sb[:, ch0:2 * ch0])
                pos[("sub", 0)] = v.n

                # part B (needs nrm = sqrt(nrm2) from ACT; aq threshold 1)
                eng.wait_ge(aq, 1)
                v.op(eng.reciprocal, rinv[:], nrm[:])
                for i in range(4):
                    v.op(eng.tensor_mul, Q[:, i, 0:_R], dr3[:, :, 1 + i],
                         rinv[:], serialize=True)
                qpv = Q[:, :, _R:W2].rearrange("p c r -> p r c")
                v.op(eng.tensor_copy, qpv, an3[:, :, 1:5], serialize=True)
                _emit_quat2mat(v, nc, st, f32,
                               [Q[:, i, :] for i in range(4)], M[:], W2)
                mg = M[:, 0:_R, :]
                mp = M[:, _R:W2, :]
                v.op(eng.tensor_sub, d1[:], mg, mp, serialize=True)
                v.op(eng.tensor_mul, d1s[:], d1[:], d1[:], serialize=True)
                v.op(eng.tensor_reduce, out=n1sq[:], in_=d1s[:], axis=AxX,
                     op=Alu.add, serialize=True)
                # m_gt - m_pred @ RY: columns 0 and 2 of m_pred flip sign
                v.op(eng.tensor_add, f2[:, :, 0:3], mg[:, :, 0:3],
                     mp[:, :, 0:3], serialize=True)
                v.op(eng.tensor_copy, f2[:, :, 3:6], d1[:, :, 3:6],
                     serialize=True)
                v.op(eng.tensor_add, f2[:, :, 6:9], mg[:, :, 6:9],
                     mp[:, :, 6:9], serialize=True)
                v.op(eng.tensor_mul, f2s[:], f2[:], f2[:], serialize=True)
                v.op(eng.tensor_reduce, out=n2sq[:], in_=f2s[:], axis=AxX,
                     op=Alu.add, serialize=True)
                v.op(eng.tensor_tensor, nminsq[:], n1sq[:], n2sq[:],
                     op=Alu.min, serialize=True)
                pos["nminsq"] = v.n

                # rotation accumulate (needs nmin = ACT op #3: sqrt1,
                # square_0, sqrt2 -> aq threshold 3)
                eng.wait_ge(aq, 3)
                v.op(eng.scalar_tensor_tensor,
                     out=rscr[:], in0=nmin[:], scalar=1.0, in1=mt,
                     op0=Alu.mult, op1=Alu.mult, serialize=True,
                     accum_out=accs[:, 3:4])

                # confidence stream, software-pipelined: sub_i ; wd2_{i-1}.
                # wd2 = w * d^2 (both TT passes run in DVE 2x bf16 mode);
                # the PE reduces wd2 columns into PSUM via ones-matmuls.
                # Ordering within a chunk's chain is transitive through the
                # cross-engine aq/vq waits; no same-engine waits needed.
                def sub(i, ch):
                    at = sb[:, 3 * offs[i]: 3 * offs[i] + ch]
                    bt = sb[:, 3 * offs[i] + ch: 3 * offs[i] + 2 * ch]
                    eng.wait_ge(dsems[i], 16)
                    v.op(eng.tensor_sub, at, at, bt)
                    pos[("sub", i)] = v.n

                def wd2(i, ch):
                    at = sb[:, 3 * offs[i]: 3 * offs[i] + ch]
                    bt = sb[:, 3 * offs[i] + ch: 3 * offs[i] + 2 * ch]
                    wt = sb[:, 3 * offs[i] + 2 * ch: 3 * offs[i] + 3 * ch]
                    if i < _NCH - _NTAIL:
                        # ACT program: sqrt1, square_0, sqrt2, square_1, ...
                        eng.wait_ge(aq, 2 if i == 0 else 3 + i)
                        v.op(eng.tensor_mul, bt, at, wt)
                    else:
                        # tail chunks: square on DVE (skip the ACT round
                        # trip so the post-stream drain chain is short)
                        v.op(eng.tensor_mul, at, at, at, serialize=True)
                        v.op(eng.tensor_mul, bt, at, wt, serialize=True)
                    pos[("wd2", i)] = v.n

                for i, ch in enumerate(_CHUNKS):
                    if i > 0:
                        sub(i, ch)     # sub_0 was emitted before ROI part B
                        wd2(i - 1, _CHUNKS[i - 1])
                wd2(_NCH - 1, _CHUNKS[-1])

                # conf totals: group-A PSUM row is complete well before the
                # stream ends; its (long) reduce overlaps the tail matmuls.
                eng.wait_ge(pq, n_mm_a)
                v.op(eng.tensor_reduce, out=accs[0:1, 0:1], in_=ps[0:1, :],
                     axis=AxX, op=Alu.add, serialize=True)
                eng.wait_ge(pq, n_mm)
                v.op(eng.tensor_reduce, out=accs[0:1, 1:2], in_=psb[0:1, :],
                     axis=AxX, op=Alu.add, serialize=True)
                pos["end"] = v.n

            @blk.scalar
            def _(eng):
                a = _Counter(eng, aq)
                # aq=1: nrm = sqrt(nrm2)
                eng.wait_ge(vq, pos["nrm2"])
                a.op(eng.activation, nrm[:], nrm2[:], Act.Sqrt)
                # aq=2: square_0 (sub_0 precedes ROI part B on DVE)
                at0 = sb[:, 0:_CHUNKS[0]]
                eng.wait_ge(vq, pos[("sub", 0)])
                a.op(eng.activation, at0, at0, Act.Square)
                # aq=3: nmin = sqrt(min(n1sq, n2sq))
                eng.wait_ge(vq, pos["nminsq"])
                a.op(eng.activation, nmin[:], nminsq[:], Act.Sqrt)
                # aq=3+i: remaining chunk squares (in-place on d=a-b);
                # the tail chunks square on DVE instead.
                for i, ch in enumerate(_CHUNKS[:-_NTAIL]):
                    if i == 0:
                        continue
                    at = sb[:, 3 * offs[i]: 3 * offs[i] + ch]
                    eng.wait_ge(vq, pos[("sub", i)])
                    a.op(eng.activation, at, at, Act.Square)

            @blk.sync
            def _(eng):
                # ROI data first: tiny (45 KB), heads the big ring so its
                # completion is not starved behind the chunk stream.
                eng.dma_start(out=rb[:], in_=roid[:]).then_inc(rsem, 16)
                if not _FP8:
                    for i, ch in enumerate(_CHUNKS):
                        s = slice(3 * offs[i], 3 * offs[i] + 3 * ch)
                        eng.dma_start(out=sb[:, s], in_=comb[:, s]).then_inc(
                            dsems[i], 16)
                eng.wait_ge(vq, pos["end"])   # all accs columns written
                eng.dma_start(out=out[:], in_=accs[:]).then_inc(fsem, 16)
                eng.wait_ge(fsem, 16)

            @blk.tensor
            def _(eng):
                p = _Counter(eng, pq)
                k = 0
                for i, ch in enumerate(_CHUNKS[:-_NTAIL]):
                    eng.wait_ge(vq, pos[("wd2", i)])
                    for j in range(0, ch, _TW):
                        w_ = min(_TW, ch - j)
                        bt = sb[:, 3 * offs[i] + ch + j:
                                3 * offs[i] + ch + j + w_]
                        p.op(eng.matmul, ps[0:1, 0:w_], ones[:, 0:1], bt,
                             start=(k == 0), stop=(k == n_mm_a - 1))
                        k += 1
                for t in range(_NTAIL):
                    i = _NCH - _NTAIL + t
                    ch = _CHUNKS[i]
                    bt = sb[:, 3 * offs[i] + ch: 3 * offs[i] + 2 * ch]
                    eng.wait_ge(vq, pos[("wd2", i)])
                    p.op(eng.matmul, psb[0:1, 0:ch], ones[:, 0:1], bt,
                         start=(t == 0), stop=(t == _NTAIL - 1))

            @blk.gpsimd
            def _(eng):
                if _FP8:
                    # SWDGE chunk DMAs: fp8 in HBM, cast to bf16 into SBUF.
                    for i, ch in enumerate(_CHUNKS):
                        s = slice(3 * offs[i], 3 * offs[i] + 3 * ch)
                        eng.dma_start(out=sb[:, s], in_=comb[:, s]).then_inc(
                            dsems[i], 16)
                else:
                    eng.nop()

        # After the Block's final all-engine barrier: reset every semaphore
        # this program used so re-executions of the NEFF start from zero
        # (raw sem allocation does NOT clear; Tile normally emits this).
        nc.clear_and_free_semaphores(all_sems)

        nc.compile()
    return nc


def _get_nc():
    if "nc" not in _CACHE:
        _CACHE["nc"] = build_nc()
    return _CACHE["nc"]


def make_in_maps(confidence, confidence_gt, weight, depth_and_rotation,
                 ann_values, ann_flags):
    import ml_dtypes
    sdt = ml_dtypes.float8_e4m3fn if _FP8 else ml_dtypes.bfloat16
    a = np.ascontiguousarray(confidence, dtype=np.float32).reshape(
        _NCORES, 128, _F).astype(sdt)
    b = np.ascontiguousarray(confidence_gt, dtype=np.float32).reshape(
        _NCORES, 128, _F).astype(sdt)
    w = np.ascontiguousarray(weight, dtype=np.float32).reshape(
        _NCORES, 128, _F).astype(sdt)
    comb = np.empty((_NCORES, 128, 3 * _F), dtype=sdt)
    o = 0
    for ch in _CHUNKS:
        comb[:, :, 3 * o: 3 * o + ch] = a[:, :, o:o + ch]
        comb[:, :, 3 * o + ch: 3 * o + 2 * ch] = b[:, :, o:o + ch]
        comb[:, :, 3 * o + 2 * ch: 3 * o + 3 * ch] = w[:, :, o:o + ch]
        o += ch
    dr = np.ascontiguousarray(depth_and_rotation, dtype=np.float32).reshape(
        _NCORES, 128, _R * 5)
    an = np.ascontiguousarray(ann_values, dtype=np.float32).reshape(
        _NCORES, 128, _R * 5)
    mk = np.ascontiguousarray(ann_flags).astype(np.float32).reshape(
        _NCORES, 128, _R)
    roi = np.concatenate([dr, an, mk], axis=2)
    return [dict(comb=comb[c], roid=roi[c]) for c in range(_NCORES)]


def reduce_outs(outs):
    """outs: list of per-core {'out': [128, 4]} -> (conf, depth, rot).
    conf partials live in out[0, 0] + out[0, 1] (PE group A/B reduces)."""
    P = np.stack([o["out"] for o in outs]).astype(np.float64)
    conf = (P[:, 0, 0] + P[:, 0, 1]).sum() / float(_HW)
    dep = P[:, :, 2].sum() / float(_N)
    rot = P[:, :, 3].sum() / float(_N)
    return (np.float32(conf), np.float32(dep), np.float32(rot))


def kernel(confidence, confidence_gt, weight, depth_and_rotation,
           ann_values, ann_flags):
    from concourse.bass_utils import run_bass_kernel_spmd
    nc = _get_nc()
    in_maps = make_in_maps(confidence, confidence_gt, weight,
                           depth_and_rotation, ann_values, ann_flags)
    res = run_bass_kernel_spmd(nc, in_maps, core_ids=list(range(_NCORES)))
    return reduce_outs(res.results)


# revision 52
# speedup vs baseline: 1.0291x; 1.0291x over previous
"""Trainium2 Bass kernel for the HPNET loss (confidence + depth + rotation).

Contract: kernel(**inputs) takes the FULL unsharded fp32 inputs and returns
the full output (tuple of three f32 scalars), distributing across 8 cores.

Strategy (raw Bass, no TileContext):
  - Data-parallel: batch dim of confidence/gt/weight and ROI dim of
    depth_and_rotation/ann_* split 8 ways; tiny [128, 3] partials per core
    ([conf (partition 0 only), depth, rot]) are reduced on host.
  - The confidence stream (a, b, w) is host-cast to fp8 e4m3 and packed into
    ONE chunk-interleaved DRAM tensor per core: per chunk [a|b|w]. SWDGE
    (gpsimd) chunk DMAs cast fp8 -> bf16 on the way into SBUF, so HBM reads
    are quartered and the stream runs at the SBUF-write fabric rate
    (~413 GB/s), which also makes the stream time run-to-run stable.
    Quantization adds ~1.2e-3 rel err vs the 2e-2 budget.
  - No SBUF buffer reuse: the whole 96 KiB/partition stream is resident, so
    every DMA is issued up-front with zero recycle waits and the queue
    streams back-to-back.
  - Compute per chunk: DVE sub (in-place, bf16 2x mode), ACT square
    (in-place), DVE w*d^2 (2x mode), then the otherwise-idle PE reduces
    wd2 columns into a PSUM row via ones-vector matmuls (accumulated across
    all chunks); one final DVE reduce of the PSUM row yields the conf sum.
    Chunk sizes taper at the end so the post-last-DMA drain is short.
  - ROI losses (fp32, tiny) ride the first DMA on the sync-engine ring
    (heading the ring avoids starvation behind the stream) and overlap the
    start of the stream.  m_gt normalization and min(n1,n2) =
    sqrt(min(n1sq,n2sq)) keep the ROI chain to two ACT ops.
  - Raw-Bass sync protocol (one sem update per instruction max): every DVE op
    increments the retire-counter sem `vq`, every ACT compute op `aq`, every
    PE matmul `pq`.  All cross-engine dependencies are thresholds on these
    counters (= producer's position in its engine's program).  Same-engine
    RAW hazards in the ROI chain are ordered by waiting on `vq` (TRN2 engines
    do not interlock back-to-back instructions); the streaming chunk ops are
    ordered transitively through their cross-engine waits.  Each chunk DMA
    gets a dedicated completion sem (+16).  All sems are cleared after the
    final barrier so NEFF re-executions start clean.
"""

import numpy as np

_NCORES = 8
_B = 256
_HW = 256 * 256
_N = 8192
_PB = _B // _NCORES            # batches per core
_F = _PB * _HW // 128          # 16384 free elems per partition
_CHUNKS = (2048, 2048, 2048, 2048, 2048, 2048, 1024, 1024, 1024,
           512, 256, 128, 128)
assert sum(_CHUNKS) == _F
_NCH = len(_CHUNKS)
_NTAIL = 2                     # last chunks: DVE-only square, PSUM group B
_R = _N // _NCORES // 128      # 8 ROIs per partition
_OUTC = 4                      # [confA, confB (partition 0 only), depth, rot]
_ROIW = _R * 5 * 2 + _R        # dr(40) + ann(40) + msk(8) = 88 f32
_TW = 256                      # PE column-sum tile width (half a PSUM bank)
_FP8 = True                    # store conf stream as fp8 e4m3 in HBM,
                               # SWDGE-cast to bf16 on the way into SBUF

_CACHE = {}


class _Counter:
    """Emit ops on one engine; every op .then_inc's the engine's retire
    counter sem. `serialize=True` additionally waits for all previously
    emitted ops on this engine (same-engine memory ordering)."""

    def __init__(self, eng, sem):
        self.eng, self.sem, self.n = eng, sem, 0

    def op(self, f, *a, serialize=False, **k):
        if serialize and self.n:
            self.eng.wait_ge(self.sem, self.n)
        ins = f(*a, **k)
        ins.then_inc(self.sem, 1)
        self.n += 1
        return ins


def _emit_quat2mat(v, nc, st, f32, q, m, width):
    """Emit rotation-matrix entries (column-major: m[:,:,3*col+row]) for
    quaternions given as 4 APs of shape [128, width]. No normalization.
    All ops serialized on the DVE stream (RAW chains throughout)."""
    eng = v.eng
    sq = st.enter_context(nc.sbuf_tensor([128, 4, width], f32))
    for i in range(4):
        v.op(eng.tensor_mul, sq[:, i, :], q[i], q[i], serialize=True)
    qd = st.enter_context(nc.sbuf_tensor([128, 3, width], f32))
    for a0 in range(3):
        v.op(eng.tensor_scalar_mul, qd[:, a0, :], q[a0], 2.0, serialize=True)
    pairs = [(0, 1), (0, 2), (0, 3), (1, 2), (1, 3), (2, 3)]
    pp = st.enter_context(nc.sbuf_tensor([128, 6, width], f32))
    for k, (x, y) in enumerate(pairs):
        v.op(eng.tensor_mul, pp[:, k, :], qd[:, x, :], q[y], serialize=True)
    uv = st.enter_context(nc.sbuf_tensor([128, 4, width], f32))
    v.op(eng.tensor_sub, uv[:, 0, :], sq[:, 0, :], sq[:, 3, :],
         serialize=True)
    v.op(eng.tensor_sub, uv[:, 1, :], sq[:, 1, :], sq[:, 2, :],
         serialize=True)
    v.op(eng.tensor_add, uv[:, 2, :], sq[:, 0, :], sq[:, 3, :],
         serialize=True)
    v.op(eng.tensor_add, uv[:, 3, :], sq[:, 1, :], sq[:, 2, :],
         serialize=True)
    P01, P02, P03, P12, P13, P23 = (pp[:, k, :] for k in range(6))
    u, vv, u2, v2 = (uv[:, k, :] for k in range(4))
    v.op(eng.tensor_add, m[:, :, 0], u, vv, serialize=True)
    v.op(eng.tensor_add, m[:, :, 1], P12, P03, serialize=True)
    v.op(eng.tensor_sub, m[:, :, 2], P13, P02, serialize=True)
    v.op(eng.tensor_sub, m[:, :, 3], P12, P03, serialize=True)
    v.op(eng.tensor_sub, m[:, :, 4], u, vv, serialize=True)
    v.op(eng.tensor_add, m[:, :, 5], P23, P01, serialize=True)
    v.op(eng.tensor_add, m[:, :, 6], P13, P02, serialize=True)
    v.op(eng.tensor_sub, m[:, :, 7], P23, P01, serialize=True)
    v.op(eng.tensor_sub, m[:, :, 8], u2, v2, serialize=True)


def build_nc():
    from contextlib import ExitStack
    import concourse.bacc as bacc
    import concourse.mybir as mybir

    f32 = mybir.dt.float32
    bf16 = mybir.dt.bfloat16
    Alu = mybir.AluOpType
    Act = mybir.ActivationFunctionType
    AxX = mybir.AxisListType.X

    nc = bacc.Bacc("TRN2", target_bir_lowering=False, debug=False,
                   num_devices=_NCORES)

    sdt = mybir.dt.float8e4 if _FP8 else bf16
    comb = nc.dram_tensor("comb", [128, 3 * _F], sdt, kind="ExternalInput")
    roid = nc.dram_tensor("roid", [128, _ROIW], f32, kind="ExternalInput")
    out = nc.dram_tensor("out", [128, _OUTC], f32, kind="ExternalOutput")

    offs = []
    o = 0
    for ch in _CHUNKS:
        offs.append(o)
        o += ch

    # DVE program positions (1-based vq thresholds), computed as we emit.
    pos = {}

    with ExitStack() as st:
        sb = st.enter_context(nc.sbuf_tensor([128, 3 * _F], bf16))
        rb = st.enter_context(nc.sbuf_tensor([128, _ROIW], f32))
        accs = st.enter_context(nc.sbuf_tensor([128, _OUTC], f32))

        # ROI scratch (all fp32, tiny)
        W2 = 2 * _R
        qsq = st.enter_context(nc.sbuf_tensor([128, _R, 4], f32))
        nrm2 = st.enter_context(nc.sbuf_tensor([128, _R], f32))
        nrm = st.enter_context(nc.sbuf_tensor([128, _R], f32))
        rinv = st.enter_context(nc.sbuf_tensor([128, _R], f32))
        Q = st.enter_context(nc.sbuf_tensor([128, 4, W2], f32))
        M = st.enter_context(nc.sbuf_tensor([128, W2, 9], f32))
        d1 = st.enter_context(nc.sbuf_tensor([128, _R, 9], f32))
        d1s = st.enter_context(nc.sbuf_tensor([128, _R, 9], f32))
        n1sq = st.enter_context(nc.sbuf_tensor([128, _R], f32))
        f2 = st.enter_context(nc.sbuf_tensor([128, _R, 9], f32))
        f2s = st.enter_context(nc.sbuf_tensor([128, _R, 9], f32))
        n2sq = st.enter_context(nc.sbuf_tensor([128, _R], f32))
        nminsq = st.enter_context(nc.sbuf_tensor([128, _R], f32))
        nmin = st.enter_context(nc.sbuf_tensor([128, _R], f32))
        dd = st.enter_context(nc.sbuf_tensor([128, _R], f32))
        dd2 = st.enter_context(nc.sbuf_tensor([128, _R], f32))
        dscr = st.enter_context(nc.sbuf_tensor([128, _R], f32))
        rscr = st.enter_context(nc.sbuf_tensor([128, _R], f32))

        ones = st.enter_context(nc.sbuf_tensor([128, 1], bf16))
        ps = st.enter_context(nc.psum_tensor([1, _TW], f32))
        psb = st.enter_context(nc.psum_tensor([1, max(_CHUNKS[-_NTAIL:])], f32))

        dsems = [nc.alloc_semaphore(f"dsem{i}") for i in range(_NCH)]
        rsem = nc.alloc_semaphore("rsem")   # ROI DMA completion
        fsem = nc.alloc_semaphore("fsem")   # out DMA done
        vq = nc.alloc_semaphore("vq")       # DVE retire counter
        aq = nc.alloc_semaphore("aq")       # ACT retire counter
        pq = nc.alloc_semaphore("pq")       # PE retire counter
        all_sems = dsems + [rsem, fsem, vq, aq, pq]

        n_mm_a = sum((ch + _TW - 1) // _TW for ch in _CHUNKS[:-_NTAIL])
        n_mm = n_mm_a + _NTAIL

        # ---- vector program (emitted first so `pos` is known to others) ----
        with nc.Block(no_gpsimd_drain=not _FP8) as blk:

            @blk.vector
            def _(eng):
                v = _Counter(eng, vq)
                dr3 = rb[:, 0:5 * _R].rearrange("p (r c) -> p r c", c=5)
                an3 = rb[:, 5 * _R:10 * _R].rearrange("p (r c) -> p r c", c=5)
                mt = rb[:, 10 * _R:11 * _R]

                v.op(eng.memset, ones[:], 1.0)   # PE stationary ones vector
                v.op(eng.memset, accs[:], 0.0)

                eng.wait_ge(rsem, 16)
                # depth loss (DVE only; serialized RAW chain)
                v.op(eng.tensor_sub, dd[:], dr3[:, :, 0], an3[:, :, 0])
                v.op(eng.tensor_mul, dd2[:], dd[:], dd[:], serialize=True)
                v.op(eng.scalar_tensor_tensor,
                     out=dscr[:], in0=dd2[:], scalar=1.0, in1=mt,
                     op0=Alu.mult, op1=Alu.mult, serialize=True,
                     accum_out=accs[:, 2:3])

                # rotation part A: |q|^2 of predicted quaternion
                v.op(eng.tensor_mul, qsq[:], dr3[:, :, 1:5], dr3[:, :, 1:5])
                v.op(eng.tensor_reduce, out=nrm2[:], in_=qsq[:], axis=AxX,
                     op=Alu.add, serialize=True)
                pos["nrm2"] = v.n

                # part B (needs nrm = sqrt(nrm2) from ACT; aq threshold 1)
                eng.wait_ge(aq, 1)
                v.op(eng.reciprocal, rinv[:], nrm[:])
                for i in range(4):
                    v.op(eng.tensor_mul, Q[:, i, 0:_R], dr3[:, :, 1 + i],
                         rinv[:], serialize=True)
                qpv = Q[:, :, _R:W2].rearrange("p c r -> p r c")
                v.op(eng.tensor_copy, qpv, an3[:, :, 1:5], serialize=True)
                _emit_quat2mat(v, nc, st, f32,
                               [Q[:, i, :] for i in range(4)], M[:], W2)
                mg = M[:, 0:_R, :]
                mp = M[:, _R:W2, :]
                v.op(eng.tensor_sub, d1[:], mg, mp, serialize=True)
                v.op(eng.tensor_mul, d1s[:], d1[:], d1[:], serialize=True)
                v.op(eng.tensor_reduce, out=n1sq[:], in_=d1s[:], axis=AxX,
                     op=Alu.add, serialize=True)
                # m_gt - m_pred @ RY: columns 0 and 2 of m_pred flip sign
                v.op(eng.tensor_add, f2[:, :, 0:3], mg[:, :, 0:3],
                     mp[:, :, 0:3], serialize=True)
                v.op(eng.tensor_copy, f2[:, :, 3:6], d1[:, :, 3:6],
                     serialize=True)
                v.op(eng.tensor_add, f2[:, :, 6:9], mg[:, :, 6:9],
                     mp[:, :, 6:9], serialize=True)
                v.op(eng.tensor_mul, f2s[:], f2[:], f2[:], serialize=True)
                v.op(eng.tensor_reduce, out=n2sq[:], in_=f2s[:], axis=AxX,
                     op=Alu.add, serialize=True)
                v.op(eng.tensor_tensor, nminsq[:], n1sq[:], n2sq[:],
                     op=Alu.min, serialize=True)
                pos["nminsq"] = v.n

                # rotation accumulate (needs nmin from ACT; aq threshold 2)
                eng.wait_ge(aq, 2)
                v.op(eng.scalar_tensor_tensor,
                     out=rscr[:], in0=nmin[:], scalar=1.0, in1=mt,
                     op0=Alu.mult, op1=Alu.mult, serialize=True,
                     accum_out=accs[:, 3:4])

                # confidence stream, software-pipelined: sub_i ; wd2_{i-1}.
                # wd2 = w * d^2 (both TT passes run in DVE 2x bf16 mode);
                # the PE reduces wd2 columns into PSUM via ones-matmuls.
                # Ordering within a chunk's chain is transitive through the
                # cross-engine aq/vq waits; no same-engine waits needed.
                def sub(i, ch):
                    at = sb[:, 3 * offs[i]: 3 * offs[i] + ch]
                    bt = sb[:, 3 * offs[i] + ch: 3 * offs[i] + 2 * ch]
                    eng.wait_ge(dsems[i], 16)
                    v.op(eng.tensor_sub, at, at, bt)
                    pos[("sub", i)] = v.n

                def wd2(i, ch):
                    at = sb[:, 3 * offs[i]: 3 * offs[i] + ch]
                    bt = sb[:, 3 * offs[i] + ch: 3 * offs[i] + 2 * ch]
                    wt = sb[:, 3 * offs[i] + 2 * ch: 3 * offs[i] + 3 * ch]
                    if i < _NCH - _NTAIL:
                        eng.wait_ge(aq, 3 + i)   # ACT square_i retired
                        v.op(eng.tensor_mul, bt, at, wt)
                    else:
                        # tail chunks: square on DVE (skip the ACT round
                        # trip so the post-stream drain chain is short)
                        v.op(eng.tensor_mul, at, at, at, serialize=True)
                        v.op(eng.tensor_mul, bt, at, wt, serialize=True)
                    pos[("wd2", i)] = v.n

                for i, ch in enumerate(_CHUNKS):
                    sub(i, ch)
                    if i > 0:
                        wd2(i - 1, _CHUNKS[i - 1])
                wd2(_NCH - 1, _CHUNKS[-1])

                # conf totals: group-A PSUM row is complete well before the
                # stream ends; its (long) reduce overlaps the tail matmuls.
                eng.wait_ge(pq, n_mm_a)
                v.op(eng.tensor_reduce, out=accs[0:1, 0:1], in_=ps[0:1, :],
                     axis=AxX, op=Alu.add, serialize=True)
                eng.wait_ge(pq, n_mm)
                v.op(eng.tensor_reduce, out=accs[0:1, 1:2], in_=psb[0:1, :],
                     axis=AxX, op=Alu.add, serialize=True)
                pos["end"] = v.n

            @blk.scalar
            def _(eng):
                a = _Counter(eng, aq)
                # aq=1: nrm = sqrt(nrm2)
                eng.wait_ge(vq, pos["nrm2"])
                a.op(eng.activation, nrm[:], nrm2[:], Act.Sqrt)
                # aq=2: nmin = sqrt(min(n1sq, n2sq))
                eng.wait_ge(vq, pos["nminsq"])
                a.op(eng.activation, nmin[:], nminsq[:], Act.Sqrt)
                # aq=3+i: chunk squares (in-place on the d=a-b slice);
                # the tail chunks square on DVE instead.
                for i, ch in enumerate(_CHUNKS[:-_NTAIL]):
                    at = sb[:, 3 * offs[i]: 3 * offs[i] + ch]
                    eng.wait_ge(vq, pos[("sub", i)])
                    a.op(eng.activation, at, at, Act.Square)

            @blk.sync
            def _(eng):
                # ROI data first: tiny (45 KB), heads the big ring so its
                # completion is not starved behind the chunk stream.
                eng.dma_start(out=rb[:], in_=roid[:]).then_inc(rsem, 16)
                if not _FP8:
                    for i, ch in enumerate(_CHUNKS):
                        s = slice(3 * offs[i], 3 * offs[i] + 3 * ch)
                        eng.dma_start(out=sb[:, s], in_=comb[:, s]).then_inc(
                            dsems[i], 16)
                eng.wait_ge(vq, pos["end"])   # all accs columns written
                eng.dma_start(out=out[:], in_=accs[:]).then_inc(fsem, 16)
                eng.wait_ge(fsem, 16)

            @blk.tensor
            def _(eng):
                p = _Counter(eng, pq)
                k = 0
                for i, ch in enumerate(_CHUNKS[:-_NTAIL]):
                    eng.wait_ge(vq, pos[("wd2", i)])
                    for j in range(0, ch, _TW):
                        w_ = min(_TW, ch - j)
                        bt = sb[:, 3 * offs[i] + ch + j:
                                3 * offs[i] + ch + j + w_]
                        p.op(eng.matmul, ps[0:1, 0:w_], ones[:, 0:1], bt,
                             start=(k == 0), stop=(k == n_mm_a - 1))
                        k += 1
                for t in range(_NTAIL):
                    i = _NCH - _NTAIL + t
                    ch = _CHUNKS[i]
                    bt = sb[:, 3 * offs[i] + ch: 3 * offs[i] + 2 * ch]
                    eng.wait_ge(vq, pos[("wd2", i)])
                    p.op(eng.matmul, psb[0:1, 0:ch], ones[:, 0:1], bt,
                         start=(t == 0), stop=(t == _NTAIL - 1))

            @blk.gpsimd
            def _(eng):
                if _FP8:
                    # SWDGE chunk DMAs: fp8 in HBM, cast to bf16 into SBUF.
                    for i, ch in enumerate(_CHUNKS):
                        s = slice(3 * offs[i], 3 * offs[i] + 3 * ch)
                        eng.dma_start(out=sb[:, s], in_=comb[:, s]).then_inc(
                            dsems[i], 16)
                else:
                    eng.nop()

        # After the Block's final all-engine barrier: reset every semaphore
        # this program used so re-executions of the NEFF start from zero
        # (raw sem allocation does NOT clear; Tile normally emits this).
        nc.clear_and_free_semaphores(all_sems)

        nc.compile()
    return nc


def _get_nc():
    if "nc" not in _CACHE:
        _CACHE["nc"] = build_nc()
    return _CACHE["nc"]


def make_in_maps(confidence, confidence_gt, weight, depth_and_rotation,
                 ann_values, ann_flags):
    import ml_dtypes
    sdt = ml_dtypes.float8_e4m3fn if _FP8 else ml_dtypes.bfloat16
    a = np.ascontiguousarray(confidence, dtype=np.float32).reshape(
        _NCORES, 128, _F).astype(sdt)
    b = np.ascontiguousarray(confidence_gt, dtype=np.float32).reshape(
        _NCORES, 128, _F).astype(sdt)
    w = np.ascontiguousarray(weight, dtype=np.float32).reshape(
        _NCORES, 128, _F).astype(sdt)
    comb = np.empty((_NCORES, 128, 3 * _F), dtype=sdt)
    o = 0
    for ch in _CHUNKS:
        comb[:, :, 3 * o: 3 * o + ch] = a[:, :, o:o + ch]
        comb[:, :, 3 * o + ch: 3 * o + 2 * ch] = b[:, :, o:o + ch]
        comb[:, :, 3 * o + 2 * ch: 3 * o + 3 * ch] = w[:, :, o:o + ch]
        o += ch
    dr = np.ascontiguousarray(depth_and_rotation, dtype=np.float32).reshape(
        _NCORES, 128, _R * 5)
    an = np.ascontiguousarray(ann_values, dtype=np.float32).reshape(
        _NCORES, 128, _R * 5)
    mk = np.ascontiguousarray(ann_flags).astype(np.float32).reshape(
        _NCORES, 128, _R)
    roi = np.concatenate([dr, an, mk], axis=2)
    return [dict(comb=comb[c], roid=roi[c]) for c in range(_NCORES)]


def reduce_outs(outs):
    """outs: list of per-core {'out': [128, 4]} -> (conf, depth, rot).
    conf partials live in out[0, 0] + out[0, 1] (PE group A/B reduces)."""
    P = np.stack([o["out"] for o in outs]).astype(np.float64)
    conf = (P[:, 0, 0] + P[:, 0, 1]).sum() / float(_HW)
    dep = P[:, :, 2].sum() / float(_N)
    rot = P[:, :, 3].sum() / float(_N)
    return (np.float32(conf), np.float32(dep), np.float32(rot))


def kernel(confidence, confidence_gt, weight, depth_and_rotation,
           ann_values, ann_flags):
    from concourse.bass_utils import run_bass_kernel_spmd
    nc = _get_nc()
    in_maps = make_in_maps(confidence, confidence_gt, weight,
                           depth_and_rotation, ann_values, ann_flags)
    res = run_bass_kernel_spmd(nc, in_maps, core_ids=list(range(_NCORES)))
    return reduce_outs(res.results)
